# revision 2
# baseline (speedup 1.0000x reference)
"""MoE (8 routed experts, top-2, + shared expert) on 8 TRN2 NeuronCores.

Strategy: generalized expert-parallel bin-packing. Host computes the gate
(fp32 numpy, exactly mirroring the reference) and packs all work into
8 cores x 2 "bins" of fixed sizes [Q1, Q2] (same NEFF on all cores).
Each bin is (token block, weight set, combine weights): the weight set is
per-bin INPUT DATA, so a bin can hold a light expert, half of a heavy
expert (split as a pair of Q2 bins), or a slice of the shared expert
(cw=1).  Bin sizes are chosen per routing instance to minimize total
padded token-units:
  h heaviest experts -> pairs of Q2 bins (Q2 = ceil(max_count/2)),
  remaining 8-h experts -> one Q1 bin each (Q1 = max light count),
  shared tokens fill all leftover bins.
For the reference routing this gives U = Q1+Q2 = 1582 token-units/core
vs 1664 for the naive max-capacity scheme (PE-bound workload).

The combine weight is applied AFTER the second GEMM (y = (H @ w2T) * cw),
folded into the PSUM->bf16 copy, which is numerically equivalent within
bf16 noise and removes a vector op from phase A.
"""

import numpy as np
import ml_dtypes

import concourse.mybir as mybir
from concourse import bacc
from concourse.tile import TileContext
from concourse import bass_utils

BF16 = mybir.dt.bfloat16
F32 = mybir.dt.float32

D = 2048          # model dim
I = 1408          # expert inter dim
E = 8             # routed experts
TOPK = 2
N_CORES = 8
DPO = D // 128    # 16 chunks of the model dim
IPO = I // 128    # 11 chunks of the inter dim

_BUILD_CACHE = {}


def _c_blocks(C):
    """Split C columns into equal-ish blocks <= 512 (PSUM bank limit)."""
    nb = -(-C // 512)
    per = -(-C // (nb * 128)) * 128
    blocks = []
    off = 0
    while off < C:
        w = min(per, C - off)
        blocks.append((off, w))
        off += w
    return blocks


def _dma_chunks(C, n):
    """Split [0, C) into n roughly equal column chunks."""
    out = []
    per = -(-C // n)
    off = 0
    while off < C:
        w = min(per, C - off)
        out.append((off, w))
        off += w
    return out


def _build(qs):
    """Per-core Bass kernel processing len(qs) bins of sizes qs, each with
    its own weight set + token block + combine weights. Same NEFF SPMD on
    all 8 cores."""
    nc = bacc.Bacc("TRN2", debug=False, enable_asserts=False,
                   num_devices=N_CORES, enable_partition_id=False)

    def din(name, shape, dt=BF16):
        return nc.dram_tensor(name, shape, dt, kind="ExternalInput").ap()

    def dout(name, shape, dt=BF16):
        return nc.dram_tensor(name, shape, dt, kind="ExternalOutput").ap()

    bins = []
    for j, q in enumerate(qs):
        bins.append({
            "q": q,
            "x": din(f"x{j}", [128, DPO, q]),           # [d_pi, d_po, c]
            "cw": din(f"cw{j}", [128, q], F32),         # combine w, replicated
            "w1": din(f"w1_{j}", [IPO, 128, D]),        # [i_blk][d_pi][d_po*128+i_c]
            "w3": din(f"w3_{j}", [IPO, 128, D]),
            "w2": din(f"w2_{j}", [DPO, 128, I]),        # [d_blk][i_pi][i_po*128+d_c]
            "y": dout(f"y{j}", [128, DPO, q]),
        })

    Silu = mybir.ActivationFunctionType.Silu

    with TileContext(nc) as tc:
        with tc.tile_pool(name="main", bufs=1) as pool, \
             tc.tile_pool(name="psum", bufs=1, space="PSUM") as pp:
            for j, b in enumerate(bins):
                CJ = b["q"]
                cbs = _c_blocks(CJ)
                first = (j == 0)
                x_sb = pool.tile([128, DPO, CJ], BF16, tag=f"x_{j}",
                                 bufs=1, name=f"x_{j}")
                cw_sb = pool.tile([128, CJ], F32, tag=f"cw_{j}", bufs=1,
                                  name=f"cw_{j}")
                w1d = b["w1"]
                w3d = b["w3"]
                w13_first = []
                wdr = []
                for wd, wn in ((w1d, "w1"), (w3d, "w3")):
                    w_sb = pool.tile([128, DPO, 128], BF16, tag="w13",
                                     bufs=6, name=f"{wn}_{j}_0")
                    w13_first.append(w_sb)
                    wdr.append(wd[0].rearrange("p (a b) -> p a b", a=DPO))
                if first:
                    # startup: land the first matmuls' operands in small
                    # chunks across many DMA queues so the PE starts ~3us
                    # in instead of ~11.5us (a full 290KB x d-slice on one
                    # queue takes ~13us).
                    for w_sb, wsrc in zip(w13_first, wdr):
                        nc.sync.dma_start(w_sb[:, 0:2, :], wsrc[:, 0:2, :])
                    for coff, cw_ in _dma_chunks(CJ, 4):
                        nc.sync.dma_start(x_sb[:, 0, coff:coff + cw_],
                                          b["x"][:, 0, coff:coff + cw_])
                    for w_sb, wsrc in zip(w13_first, wdr):
                        nc.sync.dma_start(w_sb[:, 2:4, :], wsrc[:, 2:4, :])
                    for dsl in range(1, 4):
                        for coff, cw_ in _dma_chunks(CJ, 2):
                            nc.sync.dma_start(x_sb[:, dsl, coff:coff + cw_],
                                              b["x"][:, dsl, coff:coff + cw_])
                    for w_sb, wsrc in zip(w13_first, wdr):
                        nc.sync.dma_start(w_sb[:, 4:8, :], wsrc[:, 4:8, :])
                        nc.sync.dma_start(w_sb[:, 8:16, :], wsrc[:, 8:16, :])
                    for dsl in range(4, DPO):
                        for coff, cw_ in _dma_chunks(CJ, 2):
                            nc.sync.dma_start(x_sb[:, dsl, coff:coff + cw_],
                                              b["x"][:, dsl, coff:coff + cw_])
                else:
                    for w_sb, wsrc in zip(w13_first, wdr):
                        nc.sync.dma_start(w_sb[:, 0:8, :], wsrc[:, 0:8, :])
                        nc.sync.dma_start(w_sb[:, 8:16, :], wsrc[:, 8:16, :])
                    for dsl in range(DPO):
                        nc.sync.dma_start(x_sb[:, dsl, :], b["x"][:, dsl, :])
                nc.sync.dma_start(cw_sb[:], b["cw"][:])
                H = pool.tile([128, IPO, CJ], BF16, tag=f"H_{j}",
                              bufs=1, name=f"H_{j}")

                # ---- phase A: H = silu(x@w1T) * (x@w3T) ----
                for i in range(IPO):
                    if i == 0:
                        w1_sb, w3_sb = w13_first
                    else:
                        w1_sb = pool.tile([128, DPO, 128], BF16, tag="w13",
                                          bufs=6, name=f"w1_{j}_{i}")
                        nc.sync.dma_start(
                            w1_sb[:],
                            w1d[i].rearrange("p (a b) -> p a b", a=DPO))
                        w3_sb = pool.tile([128, DPO, 128], BF16, tag="w13",
                                          bufs=6, name=f"w3_{j}_{i}")
                        nc.sync.dma_start(
                            w3_sb[:],
                            w3d[i].rearrange("p (a b) -> p a b", a=DPO))
                    p1s = []
                    p3s = []
                    for bi, (off, w) in enumerate(cbs):
                        p1s.append(pp.tile([128, w], F32, tag="ps", bufs=8,
                                           name=f"p1_{j}_{i}_{bi}"))
                        p3s.append(pp.tile([128, w], F32, tag="ps", bufs=8,
                                           name=f"p3_{j}_{i}_{bi}"))
                    for d in range(DPO):
                        for bi, (off, w) in enumerate(cbs):
                            nc.tensor.matmul(
                                p1s[bi][:], w1_sb[:, d, :],
                                x_sb[:, d, off:off + w],
                                start=(d == 0), stop=(d == DPO - 1))
                        for bi, (off, w) in enumerate(cbs):
                            nc.tensor.matmul(
                                p3s[bi][:], w3_sb[:, d, :],
                                x_sb[:, d, off:off + w],
                                start=(d == 0), stop=(d == DPO - 1))
                    for bi, (off, w) in enumerate(cbs):
                        s_t = pool.tile([128, w], F32, tag="act1", bufs=6,
                                        name=f"s_{j}_{i}_{bi}")
                        nc.scalar.activation(s_t[:], p1s[bi][:], Silu)
                        nc.vector.tensor_mul(H[:, i, off:off + w],
                                             s_t[:], p3s[bi][:])

                # ---- phase B: y = (H @ w2T) * cw ----
                for do in range(DPO):
                    w2_sb = pool.tile([128, IPO, 128], BF16, tag="w2",
                                      bufs=5, name=f"w2_{j}_{do}")
                    nc.sync.dma_start(
                        w2_sb[:],
                        b["w2"][do].rearrange("p (a b) -> p a b", a=IPO))
                    pys = []
                    for bi, (off, w) in enumerate(cbs):
                        pys.append(pp.tile([128, w], F32, tag="ps", bufs=8,
                                           name=f"py_{j}_{do}_{bi}"))
                    for i in range(IPO):
                        for bi, (off, w) in enumerate(cbs):
                            nc.tensor.matmul(
                                pys[bi][:], w2_sb[:, i, :],
                                H[:, i, off:off + w],
                                start=(i == 0), stop=(i == IPO - 1))
                    last = (j == len(bins) - 1) and (do == DPO - 1)
                    for bi, (off, w) in enumerate(cbs):
                        y_t = pool.tile([128, w], BF16, tag="yo", bufs=8,
                                        name=f"y_{j}_{do}_{bi}")
                        nc.vector.tensor_mul(y_t[:], pys[bi][:],
                                             cw_sb[:, off:off + w])
                        if last:
                            # split the tail stores across queues so the
                            # final DMA doesn't serialize ~130KB on one ring
                            for coff, cw_ in _dma_chunks(w, 4):
                                nc.sync.dma_start(
                                    b["y"][:, do, off + coff:off + coff + cw_],
                                    y_t[:, coff:coff + cw_])
                        else:
                            nc.sync.dma_start(b["y"][:, do, off:off + w],
                                              y_t[:])

    nc.finalize()
    return nc


def _get_kernel(qs):
    key = tuple(qs)
    if key not in _BUILD_CACHE:
        _BUILD_CACHE[key] = _build(key)
    return _BUILD_CACHE[key]


def _plan_bins(counts, shared_total):
    """Choose per-core bin sizes [Q1, Q2] and the bin assignment.

    Pattern: h heaviest experts as pairs of Q2 bins, the rest one Q1 bin
    each, shared tokens fill the h*Q1 + (8-2h)*Q2 leftover bins. Returns
    (Q1, Q2, h) minimizing Q1+Q2 over feasible h."""
    order = np.argsort(-np.asarray(counts), kind="stable")
    cs = [counts[e] for e in order]
    best = None
    for h in range(0, 5):
        if 8 - 2 * h < 0:
            continue
        q2 = -(-cs[0] // 2) if h > 0 else 512
        q1 = cs[h] if h < 8 else 0
        # shared capacity in leftover bins
        cap = h * q1 + (8 - 2 * h) * q2
        if cap < shared_total:
            short = shared_total - cap
            if h > 0:
                q1 += -(-short // h)
            else:
                q2 += -(-short // 8)
        u = q1 + q2
        if best is None or u < best[0]:
            best = (u, q1, q2, h)
    _, q1, q2, h = best
    return q1, q2, h, order


def _pm(a, po):
    """[N, po*128] -> partition-major [128, po, N] contiguous."""
    n = a.shape[0]
    return np.ascontiguousarray(
        a.T.reshape(po, 128, n).transpose(1, 0, 2))


def kernel(x, gate_w, gate_b, w1, w2, w3, sw1, sw2, sw3):
    bf16 = ml_dtypes.bfloat16
    x = np.asarray(x)
    gate_w = np.asarray(gate_w, dtype=np.float32)
    gate_b = np.asarray(gate_b, dtype=np.float32)
    w1 = np.asarray(w1)
    w2 = np.asarray(w2)
    w3 = np.asarray(w3)
    sw1 = np.asarray(sw1)
    sw2 = np.asarray(sw2)
    sw3 = np.asarray(sw3)

    B, S, Dx = x.shape
    assert Dx == D
    T = B * S
    xt = x.reshape(T, D)

    # ---- gate (fp32, mirrors reference: sqrt(softplus), top-2 on biased) ----
    xf = xt.astype(np.float32)
    logits = xf @ gate_w.T
    scores = np.sqrt(np.log1p(np.exp(-np.abs(logits)))
                     + np.maximum(logits, 0.0))
    biased = scores + gate_b
    idx = np.argsort(-biased, axis=1, kind="stable")[:, :TOPK]
    cw = np.zeros((T, E), dtype=np.float32)
    np.put_along_axis(cw, idx, np.take_along_axis(scores, idx, axis=1), axis=1)

    sel = np.zeros((T, E), dtype=bool)
    np.put_along_axis(sel, idx, True, axis=1)
    tok_lists = [np.nonzero(sel[:, e])[0] for e in range(E)]
    counts = [len(t) for t in tok_lists]

    Q1, Q2, h, order = _plan_bins(counts, T)
    nc = _get_kernel((Q1, Q2))

    # ---- weight layout transforms (lhsT, block-major contiguous DMAs) ----
    def wA_layout(wm):  # [I, D] -> [IPO, 128, D]
        return np.ascontiguousarray(
            wm.T.reshape(DPO, 128, IPO, 128).transpose(2, 1, 0, 3)
        ).reshape(IPO, 128, D)

    def wB_layout(wm):  # [D, I] -> [DPO, 128, I]
        return np.ascontiguousarray(
            wm.T.reshape(IPO, 128, DPO, 128).transpose(2, 1, 0, 3)
        ).reshape(DPO, 128, I)

    w_cache = {}

    def expert_weights(e):
        if e not in w_cache:
            if e == E:  # shared expert
                w_cache[e] = (wA_layout(sw1), wA_layout(sw3), wB_layout(sw2))
            else:
                w_cache[e] = (wA_layout(w1[e]), wA_layout(w3[e]),
                              wB_layout(w2[e]))
        return w_cache[e]

    # ---- build job lists: (expert, token_indices) per bin ----
    heavies = list(order[:h])
    lights = list(order[h:])
    q1_jobs = []           # bins of width Q1
    for e in lights:
        q1_jobs.append((int(e), tok_lists[e]))
    q2_jobs = []           # bins of width Q2
    for e in heavies:
        toks = tok_lists[e]
        q2_jobs.append((int(e), toks[:Q2]))
        q2_jobs.append((int(e), toks[Q2:]))
    # shared tokens fill the remaining bins
    all_tok = np.arange(T)
    pos = 0
    while len(q2_jobs) < N_CORES:
        take = min(Q2, T - pos)
        q2_jobs.append((E, all_tok[pos:pos + take]))
        pos += take
    while len(q1_jobs) < N_CORES:
        take = min(Q1, T - pos)
        q1_jobs.append((E, all_tok[pos:pos + take]))
        pos += take
    assert pos == T, f"shared tokens not fully placed: {pos} vs {T}"

    # ---- per-core input prep ----
    in_maps = []
    placements = []        # (bin_idx, expert, toks) per core for unpack
    for k in range(N_CORES):
        m = {}
        pl = []
        for j, (q, (e, toks)) in enumerate(
                zip((Q1, Q2), (q1_jobs[k], q2_jobs[k]))):
            cnt = len(toks)
            xg = np.zeros((q, D), dtype=bf16)
            xg[:cnt] = xt[toks]
            cwe = np.zeros((q,), dtype=np.float32)
            if e == E:
                cwe[:cnt] = 1.0
            else:
                cwe[:cnt] = cw[toks, e]
            wa1, wa3, wb2 = expert_weights(e)
            m[f"x{j}"] = _pm(xg, DPO)
            m[f"cw{j}"] = np.ascontiguousarray(
                np.broadcast_to(cwe[None, :], (128, q)))
            m[f"w1_{j}"] = wa1
            m[f"w3_{j}"] = wa3
            m[f"w2_{j}"] = wb2
            pl.append((j, q, cnt, toks))
        in_maps.append(m)
        placements.append(pl)

    res = bass_utils.run_bass_kernel_spmd(
        nc, in_maps, core_ids=list(range(N_CORES)))
    global LAST_RESULT
    LAST_RESULT = res

    # ---- unshard + combine (fp32 accumulate, then cast) ----
    acc = np.zeros((T, D), dtype=np.float32)
    for k in range(N_CORES):
        for (j, q, cnt, toks) in placements[k]:
            ye = res.results[k][f"y{j}"]                   # [128, DPO, q]
            ye_tok = ye.transpose(2, 1, 0).reshape(q, D)   # [c, d]
            acc[toks] += ye_tok[:cnt].astype(np.float32)
    out = acc.reshape(B, S, D)
    return out.astype(x.dtype)
